# revision 24
# baseline (speedup 1.0000x reference)
"""Trainium2 Bass kernel for nn_AttentionModule (dense transformer block).

Strategy (8 NeuronCores, tensor-parallel over heads):
  - Each core owns 2 of the 16 heads: computes Q^T/K^T/V^T projections for its
    128 output columns, full-sequence attention for its 2 heads, producing the
    normalized per-head output O^T (128 ocols x 2048 tokens).
  - AllToAll redistributes O^T from ocol-sharding to token-sharding.
  - Each core then computes the fc projection + bias + residual + LayerNorm for
    its 256 tokens and returns (256, 1024); the host concatenates.

Layouts: projections & scores run in transposed (feature x token) layout so
every matmul contracts along partitions. Softmax denominator is fused into the
PV matmul via an appended ones-column on V. Matmuls run as float32r (scores,
projections) or bf16 (attention-weights x V, fc).
"""

import sys

sys.path.insert(0, "/opt/trn_rl_repo")

import os

import numpy as np
import ml_dtypes

STAGE = float(os.environ.get("KSTAGE", "5"))  # 1=proj 2=+vp 3=+attn 4=+a2a(nocc) 5=full
USE_GPS_BCAST = os.environ.get("KBCAST", "0") == "1"
KSCALE = float(os.environ.get("KSCALE", "1"))
KREPEAT = int(os.environ.get("KREPEAT", "1"))
NDVE = int(os.environ.get("KDVEXP", "3"))  # kt-groups per qc on DVE (of 16)
SCH_A = 128 * 1.4426950408889634 / 64.0
SCH_B = 16250.4

import concourse.bass as bass
import concourse.mybir as mybir
import concourse.tile as tile
from concourse import bacc
from concourse.bass import ts
from concourse.bass_utils import run_bass_kernel_spmd
from concourse.masks import make_identity

NC_CORES = 8
N, D, H, HD = 2048, 1024, 16, 64
HPC = H // NC_CORES  # heads per core (2)
W = HPC * HD  # local output columns (128)
TOK = N // NC_CORES  # tokens per core after AllToAll (256)
KC = D // 128  # contraction chunks (8)
NQT = N // 512  # 512-wide token chunks (4)
NKT = N // 128  # 128-wide key tiles (16)

F32 = mybir.dt.float32
F32R = mybir.dt.float32r
BF16 = mybir.dt.bfloat16
I16 = mybir.dt.int16
AF = mybir.ActivationFunctionType
OP = mybir.AluOpType
AX = mybir.AxisListType


def R(ap):
    return ap.bitcast(F32R)


def build_nc():
    nc = bacc.Bacc("TRN2", target_bir_lowering=False, debug=False,
                   num_devices=NC_CORES)

    xt_d = nc.dram_tensor("xt", [D, N], BF16, kind="ExternalInput")
    wqt_d = nc.dram_tensor("wqt", [D, W], BF16, kind="ExternalInput")
    wkt_d = nc.dram_tensor("wkt", [D, W], BF16, kind="ExternalInput")
    wvt_d = nc.dram_tensor("wvt", [D, W], BF16, kind="ExternalInput")
    bq_d = nc.dram_tensor("bq", [W, 1], F32, kind="ExternalInput")
    bk_d = nc.dram_tensor("bk", [W, 1], F32, kind="ExternalInput")
    bv_d = nc.dram_tensor("bv", [W, 1], F32, kind="ExternalInput")
    wfct_d = nc.dram_tensor("wfct", [D, D], BF16, kind="ExternalInput")
    bfc_d = nc.dram_tensor("bfc", [1, D], BF16, kind="ExternalInput")
    src_d = nc.dram_tensor("src", [TOK, D], F32, kind="ExternalInput")
    out_d = nc.dram_tensor("out", [TOK, D], F32, kind="ExternalOutput")
    a2a_in_d = nc.dram_tensor("a2a_in", [NC_CORES, W, TOK], BF16)
    a2a_out_d = nc.dram_tensor("a2a_out", [NC_CORES, W, TOK], BF16)

    with tile.TileContext(nc) as tc:
        with (
            tc.tile_pool(name="const", bufs=1) as cst,
            tc.tile_pool(name="big", bufs=1) as big,
            tc.tile_pool(name="sm", bufs=2) as sm,
            tc.tile_pool(name="ptp", bufs=1) as ptp,
            tc.tile_pool(name="ln", bufs=1) as lnp,
            tc.tile_pool(name="ps", bufs=3, space="PSUM") as psp,
            tc.tile_pool(name="pspv", bufs=2, space="PSUM") as pvp,
        ):
            if KREPEAT > 1:
                with tc.For_i(0, KREPEAT, 1):
                    _emit(nc, tc, cst, big, sm, ptp, lnp, psp, pvp,
                          xt_d, wqt_d, wkt_d, wvt_d, bq_d, bk_d, bv_d,
                          wfct_d, bfc_d, src_d, out_d, a2a_in_d, a2a_out_d)
            else:
                _emit(nc, tc, cst, big, sm, ptp, lnp, psp, pvp,
                      xt_d, wqt_d, wkt_d, wvt_d, bq_d, bk_d, bv_d, wfct_d,
                      bfc_d, src_d, out_d, a2a_in_d, a2a_out_d)

    nc.finalize()
    return nc


def _emit(nc, tc, cst, big, sm, ptp, lnp, psp, pvp,
          xt_d, wqt_d, wkt_d, wvt_d, bq_d, bk_d, bv_d, wfct_d, bfc_d,
          src_d, out_d, a2a_in_d, a2a_out_d):
        if True:
            # ---- constants ----
            ident = cst.tile([128, 128], BF16)
            make_identity(nc, ident[:])
            ones_row = cst.tile([1, 128], BF16)
            nc.vector.memset(ones_row[:], 1.0)
            ones_f32 = cst.tile([1, 64], F32)
            nc.vector.memset(ones_f32[:], 1.0)
            eps_s = cst.tile([128, 1], F32)
            nc.vector.memset(eps_s[:], 1e-5)

            # ---- load weights + x^T ----
            wqts = big.tile([128, KC, W], BF16)
            wkts = big.tile([128, KC, W], BF16)
            wvts = big.tile([128, KC, W], BF16)
            nc.sync.dma_start(wqts[:], wqt_d[:].rearrange("(k p) m -> p k m", p=128))
            nc.sync.dma_start(wkts[:], wkt_d[:].rearrange("(k p) m -> p k m", p=128))
            nc.sync.dma_start(wvts[:], wvt_d[:].rearrange("(k p) m -> p k m", p=128))
            bq_s = cst.tile([W, 1], F32)
            bk_s = cst.tile([W, 1], F32)
            bv_s = cst.tile([W, 1], F32)
            nc.sync.dma_start(bq_s[:], bq_d[:])
            nc.sync.dma_start(bk_s[:], bk_d[:])
            nc.sync.dma_start(bv_s[:], bv_d[:])

            xts = big.tile([128, KC, N], BF16)
            for kc in range(KC):
                nc.sync.dma_start(xts[:, kc, :], xt_d[ts(kc, 128), :])

            # ---- projections: Q^T, K^T (fp32), V^T (bf16) ----
            qts = big.tile([W, N], BF16)
            kts = big.tile([W, N], BF16)
            vts = big.tile([W, N], BF16)
            for (wts, bias_s, dst) in ((wqts, bq_s, qts), (wkts, bk_s, kts),
                                       (wvts, bv_s, vts)):
                for tcp in range(NQT // 2):
                    ps = psp.tile([128, 1024], F32, tag="st")
                    for sub in range(2):
                        tc_i = 2 * tcp + sub
                        for kc in range(KC):
                            nc.tensor.matmul(ps[:, ts(sub, 512)], wts[:, kc, :],
                                             xts[:, kc, ts(tc_i, 512)],
                                             start=(kc == 0),
                                             stop=(kc == KC - 1))
                        nc.vector.tensor_scalar_add(dst[:, ts(tc_i, 512)],
                                                    ps[:, ts(sub, 512)],
                                                    bias_s[:])

            if STAGE < 2:
                nc.sync.dma_start(out_d[0:128, :],
                                  qts[:, 0:2048].bitcast(F32))
                nc.sync.dma_start(out_d[128:256, :],
                                  kts[:, 0:2048].bitcast(F32))
                return
            # ---- V' build: per head, V natural (tok x hd) + ones column ----
            vps = []
            for h in range(HPC):
                vp = big.tile([128, NKT, HD + 1], BF16, tag=f"vp{h}",
                              name=f"vp{h}")
                vps.append(vp)
            for kt in range(NKT):
                ptr = pvp.tile([128, 128], BF16, tag="pv", name=f"tr{kt}")
                nc.tensor.transpose(ptr[:], vts[:, ts(kt, 128)], ident[:])
                for h in range(HPC):
                    nc.vector.tensor_copy(vps[h][:, kt, 0:HD],
                                          ptr[:, 64 * h:64 * h + 64])
            for h in range(HPC):
                nc.vector.memset(vps[h][:, :, HD:HD + 1], 1.0)

            # ---- late loads (needed for fc phase) ----
            wfcts = big.tile([128, KC, D], BF16)
            nc.sync.dma_start(wfcts[:], wfct_d[:].rearrange("(k p) m -> p k m", p=128))
            bfc_s = cst.tile([1, D], BF16)
            nc.sync.dma_start(bfc_s[:], bfc_d[:])
            src_s = big.tile([128, TOK // 128, D], F32)
            nc.sync.dma_start(src_s[:], src_d[:].rearrange("(t p) f -> p t f", p=128))

            if STAGE < 3:
                nc.sync.dma_start(out_d[0:128, 0:512],
                                  vps[0][:, :, 0:64].bitcast(F32))
                nc.sync.dma_start(out_d[128:256, 0:512],
                                  vps[1][:, :, 0:64].bitcast(F32))
                return
            # ---- attention ----
            ots = big.tile([W, N], BF16)
            for qc in range(NQT):
                pt = ptp.tile([128, HPC, NKT, 512], BF16, tag="pt",
                              name=f"pt_{qc}")
                for kt in range(NKT):
                    ps = psp.tile([128, 1024], F32, tag="st")
                    for h in range(HPC):
                        nc.tensor.matmul(
                            ps[:, ts(h, 512)],
                            kts[64 * h:64 * h + 64, ts(kt, 128)],
                            qts[64 * h:64 * h + 64, ts(qc, 512)],
                            start=True, stop=True)
                    if kt % 5 == 4 and (kt // 5) < NDVE:
                        # bf16 Schraudolph exp on DVE to offload ACT
                        nc.vector.tensor_scalar(
                            out=pt[:, :, kt, :].bitcast(I16), in0=ps[:],
                            scalar1=SCH_A, scalar2=SCH_B,
                            op0=OP.mult, op1=OP.add)
                    else:
                        nc.scalar.activation(
                            pt[:, :, kt, :], ps[:], AF.Exp, scale=1.0 / HD)
                for h in range(HPC):
                    pv = pvp.tile([128, 512], F32, tag="pv")
                    for kt in range(NKT):
                        nc.tensor.matmul(pv[0:HD + 1, :], vps[h][:, kt, :],
                                         pt[:, h, kt, :],
                                         start=(kt == 0), stop=(kt == NKT - 1))
                    rec = sm.tile([1, 512], F32, tag="rec")
                    nc.vector.reciprocal(rec[:], pv[HD:HD + 1, :])
                    bc = sm.tile([64, 512], F32, tag="bc")
                    if USE_GPS_BCAST:
                        nc.gpsimd.partition_broadcast(bc[:], rec[:])
                    else:
                        nc.tensor.matmul(pv[64:128, :], ones_f32[:], rec[:],
                                         start=True, stop=True)
                        nc.vector.tensor_copy(bc[:], pv[64:128, :])
                    nc.vector.tensor_tensor(ots[64 * h:64 * h + 64, ts(qc, 512)],
                                            pv[0:HD, :], bc[:], OP.mult)

            if STAGE < 4:
                nc.sync.dma_start(out_d[0:128, :], ots[:].bitcast(F32))
                return
            # ---- AllToAll: ocol-shard -> token-shard ----
            nc.sync.dma_start(a2a_in_d[:].rearrange("j p t -> p j t"),
                              ots[:].rearrange("p (j t) -> p j t", j=NC_CORES))
            if STAGE < 5:
                nc.sync.dma_start(a2a_out_d[:], a2a_in_d[:])
            else:
                nc.gpsimd.collective_compute(
                    "AllToAll", OP.bypass,
                    replica_groups=[list(range(NC_CORES))],
                    ins=[a2a_in_d[:].opt()],
                    outs=[a2a_out_d[:].opt()])
            oa = big.tile([128, NC_CORES, TOK], BF16)
            nc.sync.dma_start(oa[:], a2a_out_d[:].rearrange("j p t -> p j t"))
            if STAGE < 4.3:
                nc.sync.dma_start(out_d[0:128, :], oa[:].bitcast(F32))
                return

            # ---- fc + bias + residual + LayerNorm ----
            for tt in range(TOK // 128):
                y_s = lnp.tile([128, D], F32, tag="y")
                psf = psp.tile([128, 1024], F32, tag="st")
                for f in range(D // 512):
                    ps = psf[:, ts(f, 512)]
                    for b in range(NC_CORES):
                        nc.tensor.matmul(ps, oa[:, b, ts(tt, 128)],
                                         wfcts[:, b, ts(f, 512)],
                                         start=(b == 0), stop=False)
                    nc.tensor.matmul(ps, ones_row[:],
                                     bfc_s[0:1, ts(f, 512)],
                                     start=False, stop=True)
                    nc.vector.tensor_tensor(y_s[:, ts(f, 512)], ps,
                                            src_s[:, tt, ts(f, 512)], OP.add)
                if STAGE < 4.6:
                    nc.sync.dma_start(out_d[ts(tt, 128), :], y_s[:])
                    continue
                # LayerNorm over the 1024 features
                red = lnp.tile([128, 1], F32, tag="red")
                nmean = lnp.tile([128, 1], F32, tag="nmean")
                ssq = lnp.tile([128, 1], F32, tag="ssq")
                std_s = lnp.tile([128, 1], F32, tag="std")
                rstd = lnp.tile([128, 1], F32, tag="rstd")
                t_s = lnp.tile([128, D], F32, tag="t")
                nc.vector.reduce_sum(red[:], y_s[:], axis=AX.X)
                nc.vector.tensor_scalar_mul(nmean[:], red[:], -1.0 / D)
                nc.vector.tensor_scalar_add(t_s[:], y_s[:], nmean[:])
                nc.scalar.activation(y_s[:], t_s[:], AF.Square,
                                     accum_out=ssq[:])
                nc.scalar.activation(std_s[:], ssq[:], AF.Sqrt, bias=eps_s[:],
                                     scale=1.0 / D)
                nc.vector.reciprocal(rstd[:], std_s[:])
                nc.vector.tensor_scalar(out=y_s[:], in0=t_s[:], scalar1=rstd[:],
                                        scalar2=KSCALE, op0=OP.mult,
                                        op1=OP.mult)
                nc.sync.dma_start(out_d[ts(tt, 128), :], y_s[:])


_NC_CACHE = None


def _get_nc():
    global _NC_CACHE
    if _NC_CACHE is None:
        _NC_CACHE = build_nc()
    return _NC_CACHE


def _build_in_maps(src, Wq, bq, Wk, bk, Wv, bv, Wfc, bfc, num_heads):
    src = np.asarray(src, dtype=np.float32)
    Wq = np.asarray(Wq, dtype=np.float32)
    Wk = np.asarray(Wk, dtype=np.float32)
    Wv = np.asarray(Wv, dtype=np.float32)
    Wfc = np.asarray(Wfc, dtype=np.float32)
    bq = np.asarray(bq, dtype=np.float32)
    bk = np.asarray(bk, dtype=np.float32)
    bv = np.asarray(bv, dtype=np.float32)
    bfc = np.asarray(bfc, dtype=np.float32)
    assert int(num_heads) == H
    B, S, D_in = src.shape
    assert (B, S, D_in) == (1, N, D)

    x = src.reshape(N, D)
    xt = np.ascontiguousarray(x.T).astype(ml_dtypes.bfloat16)
    wfct = np.ascontiguousarray(Wfc.T).astype(ml_dtypes.bfloat16)
    bfc_r = bfc.reshape(1, D).astype(ml_dtypes.bfloat16)

    in_maps = []
    for c in range(NC_CORES):
        J = slice(W * c, W * (c + 1))
        in_maps.append({
            "xt": xt,
            "wqt": np.ascontiguousarray(Wq[J, :].T).astype(ml_dtypes.bfloat16),
            "wkt": np.ascontiguousarray(Wk[J, :].T).astype(ml_dtypes.bfloat16),
            "wvt": np.ascontiguousarray(Wv[J, :].T).astype(ml_dtypes.bfloat16),
            "bq": np.ascontiguousarray(bq[J].reshape(W, 1)),
            "bk": np.ascontiguousarray(bk[J].reshape(W, 1)),
            "bv": np.ascontiguousarray(bv[J].reshape(W, 1)),
            "wfct": wfct,
            "bfc": bfc_r,
            "src": np.ascontiguousarray(x[TOK * c:TOK * (c + 1), :]),
        })
    return in_maps


def _assemble(res):
    out = np.concatenate([res.results[c]["out"] for c in range(NC_CORES)],
                         axis=0)
    return out.reshape(1, N, D)


def kernel(src, Wq, bq, Wk, bk, Wv, bv, Wfc, bfc, num_heads, **kwargs):
    in_maps = _build_in_maps(src, Wq, bq, Wk, bk, Wv, bv, Wfc, bfc, num_heads)
    nc = _get_nc()
    res = run_bass_kernel_spmd(nc, in_maps, core_ids=list(range(NC_CORES)))
    return _assemble(res)


def run_traced(src, Wq, bq, Wk, bk, Wv, bv, Wfc, bfc, num_heads, **kwargs):
    """Run with NTFF profiling; returns BassKernelResults (results + timing)."""
    in_maps = _build_in_maps(src, Wq, bq, Wk, bk, Wv, bv, Wfc, bfc, num_heads)
    nc = _get_nc()
    return run_bass_kernel_spmd(nc, in_maps, core_ids=list(range(NC_CORES)),
                                trace=True)


# revision 25
# speedup vs baseline: 1.0270x; 1.0270x over previous
"""Trainium2 Bass kernel for nn_AttentionModule (dense transformer block).

Strategy (8 NeuronCores, tensor-parallel over heads):
  - Each core owns 2 of the 16 heads: computes Q^T/K^T/V^T projections for its
    128 output columns, full-sequence attention for its 2 heads, producing the
    normalized per-head output O^T (128 ocols x 2048 tokens).
  - AllToAll redistributes O^T from ocol-sharding to token-sharding.
  - Each core then computes the fc projection + bias + residual + LayerNorm for
    its 256 tokens and returns (256, 1024); the host concatenates.

Layouts: projections & scores run in transposed (feature x token) layout so
every matmul contracts along partitions. Softmax denominator is fused into the
PV matmul via an appended ones-column on V. Matmuls run as float32r (scores,
projections) or bf16 (attention-weights x V, fc).
"""

import sys

sys.path.insert(0, "/opt/trn_rl_repo")

import os

import numpy as np
import ml_dtypes

STAGE = float(os.environ.get("KSTAGE", "5"))  # 1=proj 2=+vp 3=+attn 4=+a2a(nocc) 5=full
USE_GPS_BCAST = os.environ.get("KBCAST", "0") == "1"
KSCALE = float(os.environ.get("KSCALE", "1"))
KREPEAT = int(os.environ.get("KREPEAT", "1"))
NDVE = int(os.environ.get("KDVEXP", "3"))  # kt-groups per qc on DVE (of 16)
SCH_A = 128 * 1.4426950408889634 / 64.0
SCH_B = 16250.4

import concourse.bass as bass
import concourse.mybir as mybir
import concourse.tile as tile
from concourse import bacc
from concourse.bass import ts
from concourse.bass_utils import run_bass_kernel_spmd
from concourse.masks import make_identity

NC_CORES = 8
N, D, H, HD = 2048, 1024, 16, 64
HPC = H // NC_CORES  # heads per core (2)
W = HPC * HD  # local output columns (128)
TOK = N // NC_CORES  # tokens per core after AllToAll (256)
KC = D // 128  # contraction chunks (8)
NQT = N // 512  # 512-wide token chunks (4)
NKT = N // 128  # 128-wide key tiles (16)

F32 = mybir.dt.float32
F32R = mybir.dt.float32r
BF16 = mybir.dt.bfloat16
I16 = mybir.dt.int16
AF = mybir.ActivationFunctionType
OP = mybir.AluOpType
AX = mybir.AxisListType


def R(ap):
    return ap.bitcast(F32R)


def build_nc():
    nc = bacc.Bacc("TRN2", target_bir_lowering=False, debug=False,
                   num_devices=NC_CORES)

    xt_d = nc.dram_tensor("xt", [D, N], BF16, kind="ExternalInput")
    wqt_d = nc.dram_tensor("wqt", [D, W], BF16, kind="ExternalInput")
    wkt_d = nc.dram_tensor("wkt", [D, W], BF16, kind="ExternalInput")
    wvt_d = nc.dram_tensor("wvt", [D, W], BF16, kind="ExternalInput")
    bq_d = nc.dram_tensor("bq", [W, 1], F32, kind="ExternalInput")
    bk_d = nc.dram_tensor("bk", [W, 1], F32, kind="ExternalInput")
    bv_d = nc.dram_tensor("bv", [W, 1], F32, kind="ExternalInput")
    wfct_d = nc.dram_tensor("wfct", [D, D], BF16, kind="ExternalInput")
    bfc_d = nc.dram_tensor("bfc", [1, D], BF16, kind="ExternalInput")
    src_d = nc.dram_tensor("src", [TOK, D], F32, kind="ExternalInput")
    out_d = nc.dram_tensor("out", [TOK, D], F32, kind="ExternalOutput")
    a2a_in_d = nc.dram_tensor("a2a_in", [NC_CORES, W, TOK], BF16)
    a2a_out_d = nc.dram_tensor("a2a_out", [NC_CORES, W, TOK], BF16)

    with tile.TileContext(nc) as tc:
        with (
            tc.tile_pool(name="const", bufs=1) as cst,
            tc.tile_pool(name="big", bufs=1) as big,
            tc.tile_pool(name="sm", bufs=2) as sm,
            tc.tile_pool(name="ptp", bufs=2) as ptp,
            tc.tile_pool(name="ln", bufs=1) as lnp,
            tc.tile_pool(name="ps", bufs=3, space="PSUM") as psp,
            tc.tile_pool(name="pspv", bufs=2, space="PSUM") as pvp,
        ):
            if KREPEAT > 1:
                with tc.For_i(0, KREPEAT, 1):
                    _emit(nc, tc, cst, big, sm, ptp, lnp, psp, pvp,
                          xt_d, wqt_d, wkt_d, wvt_d, bq_d, bk_d, bv_d,
                          wfct_d, bfc_d, src_d, out_d, a2a_in_d, a2a_out_d)
            else:
                _emit(nc, tc, cst, big, sm, ptp, lnp, psp, pvp,
                      xt_d, wqt_d, wkt_d, wvt_d, bq_d, bk_d, bv_d, wfct_d,
                      bfc_d, src_d, out_d, a2a_in_d, a2a_out_d)

    nc.finalize()
    return nc


def _emit(nc, tc, cst, big, sm, ptp, lnp, psp, pvp,
          xt_d, wqt_d, wkt_d, wvt_d, bq_d, bk_d, bv_d, wfct_d, bfc_d,
          src_d, out_d, a2a_in_d, a2a_out_d):
        if True:
            # ---- constants ----
            ident = cst.tile([128, 128], BF16)
            make_identity(nc, ident[:])
            ones_row = cst.tile([1, 128], BF16)
            nc.vector.memset(ones_row[:], 1.0)
            ones_f32 = cst.tile([1, 64], F32)
            nc.vector.memset(ones_f32[:], 1.0)
            eps_s = cst.tile([128, 1], F32)
            nc.vector.memset(eps_s[:], 1e-5)

            # ---- load weights + x^T ----
            wqts = big.tile([128, KC, W], BF16)
            wkts = big.tile([128, KC, W], BF16)
            wvts = big.tile([128, KC, W], BF16)
            nc.sync.dma_start(wqts[:], wqt_d[:].rearrange("(k p) m -> p k m", p=128))
            nc.sync.dma_start(wkts[:], wkt_d[:].rearrange("(k p) m -> p k m", p=128))
            nc.sync.dma_start(wvts[:], wvt_d[:].rearrange("(k p) m -> p k m", p=128))
            bq_s = cst.tile([W, 1], F32)
            bk_s = cst.tile([W, 1], F32)
            bv_s = cst.tile([W, 1], F32)
            nc.sync.dma_start(bq_s[:], bq_d[:])
            nc.sync.dma_start(bk_s[:], bk_d[:])
            nc.sync.dma_start(bv_s[:], bv_d[:])

            xts = big.tile([128, KC, N], BF16)
            for kc in range(KC):
                nc.sync.dma_start(xts[:, kc, :], xt_d[ts(kc, 128), :])

            # ---- projections: Q^T, K^T (fp32), V^T (bf16) ----
            qts = big.tile([W, N], BF16)
            kts = big.tile([W, N], BF16)
            vts = big.tile([W, N], BF16)
            for (wts, bias_s, dst) in ((wqts, bq_s, qts), (wkts, bk_s, kts),
                                       (wvts, bv_s, vts)):
                for tcp in range(NQT // 2):
                    ps = psp.tile([128, 1024], F32, tag="st")
                    for sub in range(2):
                        tc_i = 2 * tcp + sub
                        for kc in range(KC):
                            nc.tensor.matmul(ps[:, ts(sub, 512)], wts[:, kc, :],
                                             xts[:, kc, ts(tc_i, 512)],
                                             start=(kc == 0),
                                             stop=(kc == KC - 1))
                        nc.vector.tensor_scalar_add(dst[:, ts(tc_i, 512)],
                                                    ps[:, ts(sub, 512)],
                                                    bias_s[:])

            if STAGE < 2:
                nc.sync.dma_start(out_d[0:128, :],
                                  qts[:, 0:2048].bitcast(F32))
                nc.sync.dma_start(out_d[128:256, :],
                                  kts[:, 0:2048].bitcast(F32))
                return
            # ---- V' build: per head, V natural (tok x hd) + ones column ----
            vps = []
            for h in range(HPC):
                vp = big.tile([128, NKT, HD + 1], BF16, tag=f"vp{h}",
                              name=f"vp{h}")
                vps.append(vp)
            for kt in range(NKT):
                ptr = pvp.tile([128, 128], BF16, tag="pv", name=f"tr{kt}")
                nc.tensor.transpose(ptr[:], vts[:, ts(kt, 128)], ident[:])
                for h in range(HPC):
                    nc.vector.tensor_copy(vps[h][:, kt, 0:HD],
                                          ptr[:, 64 * h:64 * h + 64])
            for h in range(HPC):
                nc.vector.memset(vps[h][:, :, HD:HD + 1], 1.0)

            # ---- late loads (needed for fc phase) ----
            wfcts = big.tile([128, KC, D], BF16)
            nc.sync.dma_start(wfcts[:], wfct_d[:].rearrange("(k p) m -> p k m", p=128))
            bfc_s = cst.tile([1, D], BF16)
            nc.sync.dma_start(bfc_s[:], bfc_d[:])
            src_s = big.tile([128, TOK // 128, D], F32)
            nc.sync.dma_start(src_s[:], src_d[:].rearrange("(t p) f -> p t f", p=128))

            if STAGE < 3:
                nc.sync.dma_start(out_d[0:128, 0:512],
                                  vps[0][:, :, 0:64].bitcast(F32))
                nc.sync.dma_start(out_d[128:256, 0:512],
                                  vps[1][:, :, 0:64].bitcast(F32))
                return
            # ---- attention ----
            ots = big.tile([W, N], BF16)
            for qc in range(NQT):
                pt = ptp.tile([128, HPC, NKT, 512], BF16, tag="pt",
                              name=f"pt_{qc}")
                for kt in range(NKT):
                    ps = psp.tile([128, 1024], F32, tag="st")
                    for h in range(HPC):
                        nc.tensor.matmul(
                            ps[:, ts(h, 512)],
                            kts[64 * h:64 * h + 64, ts(kt, 128)],
                            qts[64 * h:64 * h + 64, ts(qc, 512)],
                            start=True, stop=True)
                    if kt % 5 == 4 and (kt // 5) < NDVE:
                        # bf16 Schraudolph exp on DVE to offload ACT
                        nc.vector.tensor_scalar(
                            out=pt[:, :, kt, :].bitcast(I16), in0=ps[:],
                            scalar1=SCH_A, scalar2=SCH_B,
                            op0=OP.mult, op1=OP.add)
                    else:
                        nc.scalar.activation(
                            pt[:, :, kt, :], ps[:], AF.Exp, scale=1.0 / HD)
                for h in range(HPC):
                    pv = pvp.tile([128, 512], F32, tag="pv")
                    for kt in range(NKT):
                        nc.tensor.matmul(pv[0:HD + 1, :], vps[h][:, kt, :],
                                         pt[:, h, kt, :],
                                         start=(kt == 0), stop=(kt == NKT - 1))
                    rec = sm.tile([1, 512], F32, tag="rec")
                    nc.vector.reciprocal(rec[:], pv[HD:HD + 1, :])
                    bc = sm.tile([64, 512], F32, tag="bc")
                    if USE_GPS_BCAST:
                        nc.gpsimd.partition_broadcast(bc[:], rec[:])
                    else:
                        nc.tensor.matmul(pv[64:128, :], ones_f32[:], rec[:],
                                         start=True, stop=True)
                        nc.vector.tensor_copy(bc[:], pv[64:128, :])
                    nc.vector.tensor_tensor(ots[64 * h:64 * h + 64, ts(qc, 512)],
                                            pv[0:HD, :], bc[:], OP.mult)

            if STAGE < 4:
                nc.sync.dma_start(out_d[0:128, :], ots[:].bitcast(F32))
                return
            # ---- AllToAll: ocol-shard -> token-shard ----
            nc.sync.dma_start(a2a_in_d[:].rearrange("j p t -> p j t"),
                              ots[:].rearrange("p (j t) -> p j t", j=NC_CORES))
            if STAGE < 5:
                nc.sync.dma_start(a2a_out_d[:], a2a_in_d[:])
            else:
                nc.gpsimd.collective_compute(
                    "AllToAll", OP.bypass,
                    replica_groups=[list(range(NC_CORES))],
                    ins=[a2a_in_d[:].opt()],
                    outs=[a2a_out_d[:].opt()])
            oa = big.tile([128, NC_CORES, TOK], BF16)
            nc.sync.dma_start(oa[:], a2a_out_d[:].rearrange("j p t -> p j t"))
            if STAGE < 4.3:
                nc.sync.dma_start(out_d[0:128, :], oa[:].bitcast(F32))
                return

            # ---- fc + bias + residual + LayerNorm ----
            for tt in range(TOK // 128):
                y_s = lnp.tile([128, D], F32, tag="y")
                psf = psp.tile([128, 1024], F32, tag="st")
                for f in range(D // 512):
                    ps = psf[:, ts(f, 512)]
                    for b in range(NC_CORES):
                        nc.tensor.matmul(ps, oa[:, b, ts(tt, 128)],
                                         wfcts[:, b, ts(f, 512)],
                                         start=(b == 0), stop=False)
                    nc.tensor.matmul(ps, ones_row[:],
                                     bfc_s[0:1, ts(f, 512)],
                                     start=False, stop=True)
                    nc.vector.tensor_tensor(y_s[:, ts(f, 512)], ps,
                                            src_s[:, tt, ts(f, 512)], OP.add)
                if STAGE < 4.6:
                    nc.sync.dma_start(out_d[ts(tt, 128), :], y_s[:])
                    continue
                # LayerNorm over the 1024 features
                red = lnp.tile([128, 1], F32, tag="red")
                nmean = lnp.tile([128, 1], F32, tag="nmean")
                ssq = lnp.tile([128, 1], F32, tag="ssq")
                std_s = lnp.tile([128, 1], F32, tag="std")
                rstd = lnp.tile([128, 1], F32, tag="rstd")
                t_s = lnp.tile([128, D], F32, tag="t")
                nc.vector.reduce_sum(red[:], y_s[:], axis=AX.X)
                nc.vector.tensor_scalar_mul(nmean[:], red[:], -1.0 / D)
                nc.vector.tensor_scalar_add(t_s[:], y_s[:], nmean[:])
                nc.scalar.activation(y_s[:], t_s[:], AF.Square,
                                     accum_out=ssq[:])
                nc.scalar.activation(std_s[:], ssq[:], AF.Sqrt, bias=eps_s[:],
                                     scale=1.0 / D)
                nc.vector.reciprocal(rstd[:], std_s[:])
                nc.vector.tensor_scalar(out=y_s[:], in0=t_s[:], scalar1=rstd[:],
                                        scalar2=KSCALE, op0=OP.mult,
                                        op1=OP.mult)
                nc.sync.dma_start(out_d[ts(tt, 128), :], y_s[:])


_NC_CACHE = None


def _get_nc():
    global _NC_CACHE
    if _NC_CACHE is None:
        _NC_CACHE = build_nc()
    return _NC_CACHE


def _build_in_maps(src, Wq, bq, Wk, bk, Wv, bv, Wfc, bfc, num_heads):
    src = np.asarray(src, dtype=np.float32)
    Wq = np.asarray(Wq, dtype=np.float32)
    Wk = np.asarray(Wk, dtype=np.float32)
    Wv = np.asarray(Wv, dtype=np.float32)
    Wfc = np.asarray(Wfc, dtype=np.float32)
    bq = np.asarray(bq, dtype=np.float32)
    bk = np.asarray(bk, dtype=np.float32)
    bv = np.asarray(bv, dtype=np.float32)
    bfc = np.asarray(bfc, dtype=np.float32)
    assert int(num_heads) == H
    B, S, D_in = src.shape
    assert (B, S, D_in) == (1, N, D)

    x = src.reshape(N, D)
    xt = np.ascontiguousarray(x.T).astype(ml_dtypes.bfloat16)
    wfct = np.ascontiguousarray(Wfc.T).astype(ml_dtypes.bfloat16)
    bfc_r = bfc.reshape(1, D).astype(ml_dtypes.bfloat16)

    in_maps = []
    for c in range(NC_CORES):
        J = slice(W * c, W * (c + 1))
        in_maps.append({
            "xt": xt,
            "wqt": np.ascontiguousarray(Wq[J, :].T).astype(ml_dtypes.bfloat16),
            "wkt": np.ascontiguousarray(Wk[J, :].T).astype(ml_dtypes.bfloat16),
            "wvt": np.ascontiguousarray(Wv[J, :].T).astype(ml_dtypes.bfloat16),
            "bq": np.ascontiguousarray(bq[J].reshape(W, 1)),
            "bk": np.ascontiguousarray(bk[J].reshape(W, 1)),
            "bv": np.ascontiguousarray(bv[J].reshape(W, 1)),
            "wfct": wfct,
            "bfc": bfc_r,
            "src": np.ascontiguousarray(x[TOK * c:TOK * (c + 1), :]),
        })
    return in_maps


def _assemble(res):
    out = np.concatenate([res.results[c]["out"] for c in range(NC_CORES)],
                         axis=0)
    return out.reshape(1, N, D)


def kernel(src, Wq, bq, Wk, bk, Wv, bv, Wfc, bfc, num_heads, **kwargs):
    in_maps = _build_in_maps(src, Wq, bq, Wk, bk, Wv, bv, Wfc, bfc, num_heads)
    nc = _get_nc()
    res = run_bass_kernel_spmd(nc, in_maps, core_ids=list(range(NC_CORES)))
    return _assemble(res)


def run_traced(src, Wq, bq, Wk, bk, Wv, bv, Wfc, bfc, num_heads, **kwargs):
    """Run with NTFF profiling; returns BassKernelResults (results + timing)."""
    in_maps = _build_in_maps(src, Wq, bq, Wk, bk, Wv, bv, Wfc, bfc, num_heads)
    nc = _get_nc()
    return run_bass_kernel_spmd(nc, in_maps, core_ids=list(range(NC_CORES)),
                                trace=True)
